# revision 9
# baseline (speedup 1.0000x reference)
"""Causal self-attention (B=4, T=2048, C=1024, H=16) on 8 trn2 NeuronCores.

Sharding: batch x head-group, zero collectives. Core c handles batch b = c//2
and head-group hg = c%2 (heads hg*8 .. hg*8+8, i.e. half the heads) for ALL
2048 tokens of that batch. Each core computes Q/K/V only for its 8 heads
(no redundant projection work), runs causal attention for those heads, and
produces a PARTIAL output projection out_partial = y_local @ W_proj[hg*512 :
hg*512+512].  The host sums the two partials of each batch (the only
"collective" is a numpy add on the host).

Per-core pipeline (all matmul cost = output free-size columns, so layouts
are chosen to minimize streamed columns):
  A) QKV projection (bf16): Q^T/K^T feature-major [128, 4 head-pairs, 2048]
     (head-pair d-features packed at partition offsets 0/64); V token-major
     [128 tok, kt, 8 heads, 64+1] with a ones column for the softmax
     denominator.  196,608 PE cols.
  B) Attention, token-major O: S^T[k,q] = K^T.T Q^T per (head-pair mg,
     256-query block qb, 128-key tile kt); P^T = exp(0.125 S^T) on ScalarE;
     causal mask = bf16 multiply on the two diagonal k-tiles only (the
     second diagonal tile computes only the upper query half); O[q, d+1]
     accumulates in PSUM with lhsT=P^T chunk (128 q), rhs=V tile (65 cols).
     Normalization is a per-partition reciprocal + tensor_scalar multiply
     (no partition broadcast needed).  S 139,264 + O 70,720 PE cols.
  C) y (token-major bf16) is PE-transposed per [128,128] tile into yT
     feature-major; out_partial^T[c_out, q] = W_proj_slice^T @ yT in bf16.
     8,192 + 65,536 PE cols.
  Total ~480k PE cols ~= 200us at 2.4 GHz.

ScalarE exp (~139k lane-cols ~= 120us+overhead) dominates phase B's own PE
work, so emission interleaves A's last two token-chunks and all of C's
column-groups between B's (mg, qb) blocks to keep the PE dense while the
exp backlog drains; O is emitted one k-tile behind S/exp (software
pipelining) so PE never waits on ScalarE latency.

CC_PHASES env (timing diagnostics): "A" / "AB" / "ABC" (default) builds a
kernel truncated after that phase, with intermediate tensors DMA'd out.
"""

import os
import sys

import numpy as np

for _p in ("/opt/trn_rl_repo",):
    if os.path.isdir(_p) and _p not in sys.path:
        sys.path.insert(0, _p)

import ml_dtypes

B, T, C, H = 4, 2048, 1024, 16
HD = C // H  # 64
P = 128
CI = C // P  # 8 contraction chunks for QKV
NCORE = 8
QB = 256  # query block for S
NKT = T // P  # 16 k-tiles
HL = 8  # local heads per core
CL = HL * HD  # 512 local y features
BF16 = ml_dtypes.bfloat16

_CACHE = {}
LAST_RESULTS = None


def _build():
    """Build + compile the (single, uniform) bass module once."""
    from contextlib import ExitStack

    import concourse.bass as bass  # noqa: F401
    import concourse.mybir as mybir
    from concourse import bacc, masks, tile

    dt = mybir.dt
    f32, bf16 = dt.float32, dt.bfloat16
    EXP = mybir.ActivationFunctionType.Exp

    phases = os.environ.get("CC_PHASES", "ABC")
    bx = os.environ.get("CC_BX", "smon")  # B sub-steps: s(+exp), m(ask), o, n(ormalize)
    repeat = int(os.environ.get("CC_REPEAT", "1"))
    nc = bacc.Bacc(
        "TRN2",
        target_bir_lowering=False,
        debug=False,
        enable_asserts=False,
        num_devices=NCORE,
    )
    xt = nc.dram_tensor("xt", [C, T], bf16, kind="ExternalInput").ap()
    wa = nc.dram_tensor("wa", [C, 3 * CL], bf16, kind="ExternalInput").ap()
    wp = nc.dram_tensor("wp", [CL, C], bf16, kind="ExternalInput").ap()
    mk = nc.dram_tensor("mk", [P, QB], bf16, kind="ExternalInput").ap()
    if phases == "ABC":
        out_t = nc.dram_tensor("out_t", [C, T], bf16, kind="ExternalOutput").ap()
    elif phases == "AB":
        out_y = nc.dram_tensor("out_y", [P, NKT * CL], bf16, kind="ExternalOutput").ap()
    else:
        out_k = nc.dram_tensor("out_k", [P, 4 * T], bf16, kind="ExternalOutput").ap()
        out_q = nc.dram_tensor("out_q", [P, 4 * T], bf16, kind="ExternalOutput").ap()
        out_v = nc.dram_tensor(
            "out_v", [P, NKT * HL * (HD + 1)], bf16, kind="ExternalOutput"
        ).ap()

    with tile.TileContext(nc) as tc, ExitStack() as ctx:
      for _rep in range(repeat):
            rep_ctx = ctx if repeat == 1 else ExitStack()
            res = rep_ctx.enter_context(tc.tile_pool(name="res", bufs=1))
            inp = rep_ctx.enter_context(tc.tile_pool(name="inp", bufs=1))
            KT = res.tile([P, 4, T], bf16, name="KT")
            QT = res.tile([P, 4, T], bf16, name="QT")
            V = res.tile([P, NKT, HL, HD + 1], bf16, name="Vt")
            Y = res.tile([P, NKT, CL], bf16, name="Y")
            yT = res.tile([P, 4, T], bf16, name="yT")
            mask = res.tile([P, QB], bf16, name="mask")
            ident = res.tile([P, P], bf16, name="ident")
            wp_sb = res.tile([P, 4, C], bf16, name="wp_sb")
            xt_sb = inp.tile([P, CI, T], bf16, name="xt_sb")
            wa_sb = inp.tile([P, CI, 3 * CL], bf16, name="wa_sb")

            # Input DMAs, chunked so the first A matmuls start early.
            nc.sync.dma_start(mask, mk)
            nc.sync.dma_start(
                wa_sb[:, :, :CL],
                wa[:, :CL].rearrange("(o p) f -> p o f", p=P),
            )
            xt_r = xt.rearrange("(o p) t -> p o t", p=P)
            nc.sync.dma_start(xt_sb[:, :, :512], xt_r[:, :, :512])
            nc.sync.dma_start(
                wa_sb[:, :, CL:],
                wa[:, CL:].rearrange("(o p) f -> p o f", p=P),
            )
            nc.sync.dma_start(xt_sb[:, :, 512:1024], xt_r[:, :, 512:1024])
            nc.sync.dma_start(xt_sb[:, :, 1024:], xt_r[:, :, 1024:])
            nc.sync.dma_start(wp_sb, wp.rearrange("(o p) f -> p o f", p=P))
            masks.make_identity(nc, ident)
            nc.gpsimd.memset(V[:, :, :, HD:], 1.0)
            if "n" not in bx:
                nc.gpsimd.memset(Y, 0.0)

            psS = rep_ctx.enter_context(
                tc.tile_pool(name="psS", bufs=2, space="PSUM")
            )
            psO = rep_ctx.enter_context(
                tc.tile_pool(name="psO", bufs=1, space="PSUM")
            )
            psM = rep_ctx.enter_context(
                tc.tile_pool(name="psM", bufs=2, space="PSUM")
            )
            pP = rep_ctx.enter_context(tc.tile_pool(name="pP", bufs=20))
            pR = rep_ctx.enter_context(tc.tile_pool(name="pR", bufs=8))
            osb = rep_ctx.enter_context(tc.tile_pool(name="osb", bufs=3))

            def emit_a(tb, g, filler=False):
                """One A unit: feature group g (0-3 Q, 4-7 K, 8-11 V) for
                token chunk tb (512 tokens)."""
                ps = psM.tile([P, 512], f32, name="psM_t")
                if g < 8:
                    for ci in range(CI):
                        nc.tensor.matmul(
                            ps,
                            lhsT=wa_sb[:, ci, g * P:(g + 1) * P],
                            rhs=xt_sb[:, ci, tb * 512:(tb + 1) * 512],
                            start=(ci == 0),
                            stop=(ci == CI - 1),
                        )
                    dst = QT if g < 4 else KT
                    eng = nc.vector.tensor_copy if filler else nc.scalar.copy
                    eng(dst[:, g % 4, tb * 512:(tb + 1) * 512], ps)
                else:
                    kt = tb * 4 + (g - 8)
                    for ci in range(CI):
                        nc.tensor.matmul(
                            ps,
                            lhsT=xt_sb[:, ci, kt * P:(kt + 1) * P],
                            rhs=wa_sb[:, ci, 2 * CL:3 * CL],
                            start=(ci == 0),
                            stop=(ci == CI - 1),
                        )
                    nc.vector.tensor_copy(
                        V[:, kt, :, :HD], ps.rearrange("p (h d) -> p h d", d=HD)
                    )

            def emit_c(qch, co):
                """One C unit: out_partial^T rows [co*128,(co+1)*128) for
                query chunk qch (512 queries)."""
                ps = psM.tile([P, 512], f32, name="psM_t")
                for ci in range(4):
                    nc.tensor.matmul(
                        ps,
                        lhsT=wp_sb[:, ci, co * P:(co + 1) * P],
                        rhs=yT[:, ci, qch * 512:(qch + 1) * 512],
                        start=(ci == 0),
                        stop=(ci == 3),
                    )
                o_sb = osb.tile([P, 512], bf16, name="o_sb")
                nc.vector.tensor_copy(o_sb, ps)
                nc.sync.dma_start(out_r[:, co, qch * 512:(qch + 1) * 512], o_sb)

            def emit_t(qj):
                """Transpose the [128 q, 512 c] row qj of Y into yT."""
                ps = psM.tile([P, 512], f32, name="psM_t")
                pstb = ps.bitcast(bf16)
                for g in range(4):
                    nc.tensor.matmul(
                        pstb[:, g * P:(g + 1) * P],
                        Y[:, qj, g * P:(g + 1) * P],
                        ident,
                        is_transpose=True,
                    )
                nc.vector.tensor_copy(
                    yT[:, :, qj * P:(qj + 1) * P],
                    pstb[:, :512].rearrange("p (g i) -> p g i", g=4),
                )

            def emit_o(mg, qb, kt, pt, o_blk, half):
                nkt = 2 * qb + 2
                stop_kt = nkt - 2 if half == 0 else nkt - 1
                if kt > stop_kt:
                    return
                for hh in range(2):
                    nc.tensor.matmul(
                        o_blk[:, hh, :HD + 1],
                        lhsT=pt[:, hh, half * P:(half + 1) * P],
                        rhs=V[:, kt, 2 * mg + hh, :],
                        start=(kt == 0),
                        stop=(kt == stop_kt),
                    )

            def emit_norm(mg, qb, half, o_blk):
                qj = 2 * qb + half
                for hh in range(2):
                    r = pR.tile([P, 1], f32, name="r_sb")
                    nc.vector.reciprocal(r, o_blk[:, hh, HD:HD + 1])
                    nc.vector.tensor_scalar_mul(
                        Y[:, qj, (2 * mg + hh) * HD:(2 * mg + hh + 1) * HD],
                        o_blk[:, hh, :HD],
                        r,
                    )

            def emit_b_block(mg, qb, fillers):
                """S/exp for head-pair mg, query block qb; O in two passes
                over the query halves (2 PSUM-bank accumulators each);
                `fillers` run between the passes, hiding normalize latency.

                All matmul PSUM targets sit at bank bases: S^T pairs go to
                [:, hh, :] of a 2-bank tile, the second-diagonal half-tile
                to columns [:P] of its bank (exp remaps it to pt cols P:)."""
                nkt = 2 * qb + 2
                o_blk = psO.tile([P, 2, 512], f32, name="o_blk")
                pts = []
                for kt in range(nkt):
                    first_diag = kt == nkt - 2
                    second_diag = kt == nkt - 1
                    pt = pP.tile([P, 2, QB], bf16, name="pt")
                    s2 = psS.tile([P, 2, 512], f32, name="s2")
                    if second_diag:
                        for hh in range(2):
                            hp = hh * HD
                            nc.tensor.matmul(
                                s2[:, hh, :P],
                                lhsT=KT[hp:hp + HD, mg, kt * P:(kt + 1) * P],
                                rhs=QT[hp:hp + HD, mg, qb * QB + P:(qb + 1) * QB],
                                start=True,
                                stop=True,
                            )
                        if "x" in bx:
                            nc.vector.tensor_copy(pt[:, :, P:], s2[:, :, :P])
                        else:
                            nc.scalar.activation(
                                pt[:, :, P:], s2[:, :, :P], EXP, scale=0.125
                            )
                        if "m" in bx:
                            for hh in range(2):
                                nc.vector.tensor_mul(
                                    pt[:, hh, P:], pt[:, hh, P:], mask[:, :P]
                                )
                    else:
                        for hh in range(2):
                            hp = hh * HD
                            nc.tensor.matmul(
                                s2[:, hh, :QB],
                                lhsT=KT[hp:hp + HD, mg, kt * P:(kt + 1) * P],
                                rhs=QT[hp:hp + HD, mg, qb * QB:(qb + 1) * QB],
                                start=True,
                                stop=True,
                            )
                        if "x" in bx:
                            nc.vector.tensor_copy(pt, s2[:, :, :QB])
                        else:
                            nc.scalar.activation(pt, s2[:, :, :QB], EXP, scale=0.125)
                        if first_diag and "m" in bx:
                            for hh in range(2):
                                nc.vector.tensor_mul(
                                    pt[:, hh, :], pt[:, hh, :], mask
                                )
                    pts.append(pt)
                    if "o" in bx and kt >= 1:
                        emit_o(mg, qb, kt - 1, pts[kt - 1], o_blk, half=0)
                if "o" not in bx:
                    for fill in fillers:
                        fill()
                    return
                if "n" in bx:
                    emit_norm(mg, qb, 0, o_blk)
                for fill in fillers:
                    fill()
                o_blk2 = psO.tile([P, 2, 512], f32, name="o_blk")
                for kt in range(nkt):
                    emit_o(mg, qb, kt, pts[kt], o_blk2, half=1)
                if "n" in bx:
                    emit_norm(mg, qb, 1, o_blk2)

            # ---------------- Phase A prelude: token chunks 0-1 -------------
            ntb_pre = 4 if phases == "A" else 2
            for tb in range(ntb_pre):
                for g in range(12):
                    emit_a(tb, g)

            if phases == "A":
                nc.sync.dma_start(out_k, KT.rearrange("p a b -> p (a b)"))
                nc.sync.dma_start(out_q, QT.rearrange("p a b -> p (a b)"))
                nc.sync.dma_start(out_v, V.rearrange("p a b c -> p (a b c)"))
            else:
                if phases == "ABC":
                    out_r = out_t.rearrange("(o p) q -> p o q", p=P)

                # filler schedule: (qb, mg) -> list of closures
                fills = {(qb, mg): [] for qb in range(8) for mg in range(4)}
                a_units = [(2, g) for g in range(12)] + [(3, g) for g in range(12)]
                # A tb2 over qb1..qb3 (11 slots -> 12 units), tb3 over qb4..qb5
                slots = [(1, 1), (1, 2), (1, 3)] + \
                    [(qb, mg) for qb in (2, 3) for mg in range(4)] + \
                    [(4, mg) for mg in range(4)] + \
                    [(5, mg) for mg in range(4)]
                ai = 0
                for si, slot in enumerate(slots):
                    n = 2 if si >= len(slots) - 5 else 1
                    for _ in range(n):
                        if ai < len(a_units):
                            tb, g = a_units[ai]
                            fills[slot].append(
                                lambda tb=tb, g=g: emit_a(tb, g, filler=True)
                            )
                            ai += 1
                assert ai == len(a_units)
                if phases == "ABC":
                    for mg in range(4):  # C qch0 over qb4+qb5
                        fills[(4, mg)].append(lambda co=mg: emit_c(0, co))
                        fills[(5, mg)].append(lambda co=4 + mg: emit_c(0, co))
                    for mg in range(4):  # C qch1 over qb6, qch2 over qb7
                        fills[(6, mg)].append(lambda co=2 * mg: emit_c(1, co))
                        fills[(6, mg)].append(
                            lambda co=2 * mg + 1: emit_c(1, co)
                        )
                        fills[(7, mg)].append(lambda co=2 * mg: emit_c(2, co))
                        fills[(7, mg)].append(
                            lambda co=2 * mg + 1: emit_c(2, co)
                        )

                for qb in range(8):
                    for mg in range(4):
                        emit_b_block(mg, qb, fills[(qb, mg)])
                    for half in range(2):
                        emit_t(2 * qb + half)

                if phases == "AB":
                    nc.sync.dma_start(out_y, Y.rearrange("p a b -> p (a b)"))
                else:
                    for co in range(8):
                        emit_c(3, co)

            if repeat != 1:
                rep_ctx.close()

    nc.compile()
    return nc


def _prep_inputs(x, W_attn, W_proj):
    """Host-side shard/layout prep. Pure data movement + dtype casts."""
    x = np.asarray(x, dtype=np.float32)
    W_attn = np.asarray(W_attn, dtype=np.float32)
    W_proj = np.asarray(W_proj, dtype=np.float32)
    # Causal mask in S^T layout [k within tile, q within block]: valid iff
    # k_global <= q_global  <->  kk <= i.  Same tile for every diagonal block.
    kk = np.arange(P)[:, None]
    ii = np.arange(QB)[None, :]
    mask = (kk <= ii).astype(np.float32).astype(BF16)

    in_maps = []
    for c in range(NCORE):
        b, hg = c // 2, c % 2
        xt_bf = np.ascontiguousarray(x[b].T).astype(BF16)
        sl = slice(hg * CL, (hg + 1) * CL)
        wa_bf = np.ascontiguousarray(
            np.concatenate(
                [W_attn[:, sl], W_attn[:, C:][:, sl], W_attn[:, 2 * C:][:, sl]],
                axis=1,
            )
        ).astype(BF16)
        wp_bf = np.ascontiguousarray(W_proj[sl, :]).astype(BF16)
        in_maps.append({"xt": xt_bf, "wa": wa_bf, "wp": wp_bf, "mk": mask})
    return in_maps


def kernel(x, W_attn, W_proj):
    global LAST_RESULTS
    from concourse.bass_utils import run_bass_kernel_spmd

    if "nc" not in _CACHE:
        _CACHE["nc"] = _build()
    nc = _CACHE["nc"]

    in_maps = _prep_inputs(x, W_attn, W_proj)
    trace = os.environ.get("CC_TRACE", "0") == "1"
    res = run_bass_kernel_spmd(nc, in_maps, core_ids=list(range(NCORE)), trace=trace)
    LAST_RESULTS = res

    out = np.empty((B, T, C), dtype=np.float32)
    for b in range(B):
        p0 = res.results[2 * b]["out_t"].astype(np.float32)
        p1 = res.results[2 * b + 1]["out_t"].astype(np.float32)
        out[b] = (p0 + p1).T
    return out
